# revision 2
# baseline (speedup 1.0000x reference)
"""Trainium2 Bass kernel for LstmCellWithProjection (B=64,T=256,I=512,H=512,C=4096).

Strategy: 8-way tensor-parallel over the 4*C gate dimension (each core owns a
C/8=512 column slice of each of the i/f/g/o gates, so the cell state c stays
fully local). Everything on-chip lives in a [gate_rows_on_partitions, batch]
layout so no transposes are ever needed:

  - recurrent matmul: out[g,b] += W_stateT_chunk.T @ hT_chunk   (64 chunk MMs)
  - projection:       out[h,b] += W_projT_chunk.T @ s_chunk     (16 chunk MMs)
  - per-step AllReduce (fp32, 128x256) combines the 8 partial projections.

The input projection x @ W_in.T (+bias) is a separate big GEMM over (t,b)
blocks, emitted interleaved with the recurrence so it fills the PE gaps while
the AllReduce is in flight; results live in an SBUF ring (no DRAM staging).

Ragged lengths are specialized at trace time: y is only written for the
active batch prefix A(t) (outputs are zero-initialized by the runtime), and
final h/c are captured with static slices at the steps where sequences end.
"""

import sys

sys.path.insert(0, "/opt/trn_rl_repo")

import numpy as np
import ml_dtypes

B, T, I, H, C = 64, 256, 512, 512, 4096
NCORES = 8
CLOC = C // NCORES            # 512 gate columns per gate per core
GT = 16                       # 16 gate tiles of 128 rows per core (4 gates x 4)
CLIP = 3.0

_CACHE = {}


def _build(Tmax, A):
    import concourse.bass as bass
    import concourse.tile as tile
    from concourse import bacc, mybir

    f32 = mybir.dt.float32
    bf16 = mybir.dt.bfloat16
    AF = mybir.ActivationFunctionType
    OP = mybir.AluOpType

    nc = bacc.Bacc("TRN2", target_bir_lowering=False, debug=False,
                   enable_asserts=False, num_devices=NCORES)

    wst_d = nc.dram_tensor("wst", [128, 64, 128], bf16, kind="ExternalInput")
    win_d = nc.dram_tensor("win", [128, 64, 128], bf16, kind="ExternalInput")
    wpt_d = nc.dram_tensor("wpt", [128, 16, 128], bf16, kind="ExternalInput")
    bias_d = nc.dram_tensor("bias", [128, 16], f32, kind="ExternalInput")
    xt_d = nc.dram_tensor("xt", [128, 4, T * 64], bf16, kind="ExternalInput")
    y_d = nc.dram_tensor("y", [T, 128, 4, 64], f32, kind="ExternalOutput")
    hf_d = nc.dram_tensor("hf", [128, 4, 64], f32, kind="ExternalOutput")
    cf_d = nc.dram_tensor("cf", [128, 4, 64], f32, kind="ExternalOutput")

    nblk = (Tmax + 7) // 8
    rings = {}

    with tile.TileContext(nc) as tc:
        with (
            tc.tile_pool(name="wpool", bufs=1) as wpool,
            tc.tile_pool(name="state", bufs=1) as state,
            tc.tile_pool(name="ring", bufs=4) as ringp,
            tc.tile_pool(name="xtile", bufs=3) as xtp,
            tc.tile_pool(name="zb", bufs=2) as zb,
            tc.tile_pool(name="tmp", bufs=2) as tmp,
            tc.tile_pool(name="zps", bufs=1, space="PSUM") as zps,
            tc.tile_pool(name="pps", bufs=2, space="PSUM") as pps,
            tc.tile_pool(name="gps", bufs=2, space="PSUM") as gps,
            tc.tile_pool(name="drb", bufs=2, space="DRAM") as drb,
        ):
            wst = wpool.tile([128, 64, 128], bf16, tag="wst")
            win = wpool.tile([128, 64, 128], bf16, tag="win")
            wpt = wpool.tile([128, 16, 128], bf16, tag="wpt")
            bias = wpool.tile([128, 16], f32, tag="bias")
            nc.sync.dma_start(wst[:], wst_d[:])
            nc.sync.dma_start(win[:], win_d[:])
            nc.sync.dma_start(wpt[:], wpt_d[:])
            nc.sync.dma_start(bias[:], bias_d[:])

            c_sb = state.tile([128, 4, 64], f32, tag="c")
            hT = state.tile([128, 4, 64], bf16, tag="hT")
            hF = state.tile([128, 4, 64], f32, tag="hF")
            cF = state.tile([128, 4, 64], f32, tag="cF")
            nc.vector.memset(c_sb[:], 0.0)
            nc.vector.memset(hT[:], 0.0)
            nc.vector.memset(hF[:], 0.0)
            nc.vector.memset(cF[:], 0.0)

            def emit_gemm(blk):
                t0 = 8 * blk
                ncols = min(512, (Tmax - t0) * 64)
                ring_t = ringp.tile([128, 16, 512], bf16, tag="ring")
                rings[blk] = ring_t
                xt_t = xtp.tile([128, 4, 512], bf16, tag="xt")
                for k in range(4):
                    nc.sync.dma_start(xt_t[:, k, :ncols],
                                      xt_d[:, k, t0 * 64:t0 * 64 + ncols])
                for gt in range(GT):
                    ps = gps.tile([128, 512], f32, tag="gps")
                    for k in range(4):
                        nc.tensor.matmul(ps[:, :ncols], win[:, k * 16 + gt, :],
                                         xt_t[:, k, :ncols],
                                         start=(k == 0), stop=(k == 3))
                    nc.vector.tensor_scalar(ring_t[:, gt, :ncols], ps[:, :ncols],
                                            bias[:, gt:gt + 1], None, OP.add)

            emit_gemm(0)
            if nblk > 1:
                emit_gemm(1)

            for t in range(Tmax):
                if t % 8 == 0 and t // 8 + 2 < nblk:
                    emit_gemm(t // 8 + 2)
                blk, dt = t // 8, t % 8
                ring_t = rings[blk]

                Z = zps.tile([128, 16, 64], f32, tag="Z")
                for gt in range(GT):
                    for k in range(4):
                        nc.tensor.matmul(Z[:, gt, :], wst[:, k * 16 + gt, :],
                                         hT[:, k, :],
                                         start=(k == 0), stop=(k == 3))

                z_sb = zb.tile([128, 16, 64], f32, tag="z")
                a_sb = zb.tile([128, 16, 64], f32, tag="a")
                # z = psum + (x@W_in.T + b) ; i,f gates then g then o
                nc.vector.tensor_add(z_sb[:, 0:8, :], Z[:, 0:8, :],
                                     ring_t[:, 0:8, dt * 64:dt * 64 + 64])
                nc.vector.tensor_add(z_sb[:, 8:12, :], Z[:, 8:12, :],
                                     ring_t[:, 8:12, dt * 64:dt * 64 + 64])
                nc.vector.tensor_add(z_sb[:, 12:16, :], Z[:, 12:16, :],
                                     ring_t[:, 12:16, dt * 64:dt * 64 + 64])
                nc.scalar.activation(a_sb[:, 0:8, :], z_sb[:, 0:8, :], AF.Sigmoid)
                nc.scalar.activation(a_sb[:, 8:12, :], z_sb[:, 8:12, :], AF.Tanh)
                nc.scalar.activation(a_sb[:, 12:16, :], z_sb[:, 12:16, :], AF.Sigmoid)

                t1 = tmp.tile([128, 4, 64], f32, tag="t1")
                t2 = tmp.tile([128, 4, 64], f32, tag="t2")
                t3 = tmp.tile([128, 4, 64], f32, tag="t3")
                nc.vector.tensor_mul(t1[:], a_sb[:, 0:4, :], a_sb[:, 8:12, :])
                nc.vector.tensor_mul(t2[:], a_sb[:, 4:8, :], c_sb[:])
                nc.vector.tensor_add(t3[:], t1[:], t2[:])
                nc.vector.tensor_scalar(c_sb[:], t3[:], CLIP, -CLIP, OP.min, OP.max)
                tc_t = tmp.tile([128, 4, 64], f32, tag="tc")
                nc.scalar.activation(tc_t[:], c_sb[:], AF.Tanh)
                s_bf = tmp.tile([128, 4, 64], bf16, tag="s")
                nc.vector.tensor_mul(s_bf[:], a_sb[:, 12:16, :], tc_t[:])

                hP = pps.tile([128, 4, 64], f32, tag="hP")
                for m in range(4):
                    for k in range(4):
                        nc.tensor.matmul(hP[:, m, :], wpt[:, k * 4 + m, :],
                                         s_bf[:, k, :],
                                         start=(k == 0), stop=(k == 3))

                bin_t = drb.tile([128, 4, 64], f32, tag="bin")
                bout_t = drb.tile([128, 4, 64], f32, tag="bout")
                send = tmp.tile([128, 4, 64], f32, tag="send")
                nc.vector.tensor_copy(send[:], hP[:])
                nc.sync.dma_start(bin_t[:], send[:])
                nc.gpsimd.collective_compute(
                    "AllReduce", mybir.AluOpType.add,
                    replica_groups=[list(range(NCORES))],
                    ins=[bin_t.opt()], outs=[bout_t.opt()],
                )
                recv = tmp.tile([128, 4, 64], f32, tag="recv")
                nc.sync.dma_start(recv[:], bout_t[:])

                h_sb = tmp.tile([128, 4, 64], f32, tag="h")
                nc.vector.tensor_scalar(h_sb[:], recv[:], CLIP, -CLIP, OP.min, OP.max)
                nc.vector.tensor_scalar(hT[:], recv[:], CLIP, -CLIP, OP.min, OP.max)

                At = A[t]
                nc.sync.dma_start(y_d[t, :, :, 0:At], h_sb[:, :, 0:At])
                lo, hi = A[t + 1], A[t]
                if lo < hi:
                    nc.vector.tensor_copy(hF[:, :, lo:hi], h_sb[:, :, lo:hi])
                    nc.vector.tensor_copy(cF[:, :, lo:hi], c_sb[:, :, lo:hi])

            nc.sync.dma_start(hf_d[:], hF[:])
            nc.sync.dma_start(cf_d[:], cF[:])

    nc.compile()
    return nc


def _prep_inputs(inputs, batch_lengths, W_in, W_state, b_state, W_proj):
    x = np.asarray(inputs, dtype=np.float32)
    W_in = np.asarray(W_in, dtype=np.float32)
    W_state = np.asarray(W_state, dtype=np.float32)
    b_state = np.asarray(b_state, dtype=np.float32)
    W_proj = np.asarray(W_proj, dtype=np.float32)

    # xt[p, k, t*64+b] = x[b, t, k*128+p]
    xt = x.transpose(2, 1, 0).reshape(4, 128, T * 64).transpose(1, 0, 2)
    xt = np.ascontiguousarray(xt).astype(ml_dtypes.bfloat16)

    in_maps = []
    for j in range(NCORES):
        rows = np.concatenate(
            [q * C + j * CLOC + np.arange(CLOC) for q in range(4)])
        wst_np = W_state[rows, :].T          # [512 h, 2048 g]
        wst = wst_np.reshape(4, 128, 16, 128).transpose(1, 0, 2, 3)
        win_np = W_in[rows, :].T
        win = win_np.reshape(4, 128, 16, 128).transpose(1, 0, 2, 3)
        bias = b_state[rows].reshape(16, 128).T   # [128, 16]
        wp_np = W_proj[:, j * CLOC:(j + 1) * CLOC].T   # [512 c, 512 h]
        wpt = wp_np.reshape(4, 128, 4, 128).transpose(1, 0, 2, 3)
        in_maps.append({
            "wst": np.ascontiguousarray(wst).astype(ml_dtypes.bfloat16),
            "win": np.ascontiguousarray(win).astype(ml_dtypes.bfloat16),
            "wpt": np.ascontiguousarray(wpt).astype(ml_dtypes.bfloat16),
            "bias": np.ascontiguousarray(bias, dtype=np.float32),
            "xt": xt,
        })
    return in_maps


def kernel(inputs, batch_lengths, W_in, W_state, b_state, W_proj):
    from concourse.bass_utils import run_bass_kernel_spmd

    lengths = np.asarray(batch_lengths).astype(np.int64)
    Tmax = int(lengths.max())
    A = [int((lengths > t).sum()) for t in range(Tmax + 1)]

    key = (Tmax, tuple(A))
    if key not in _CACHE:
        _CACHE[key] = _build(Tmax, A)
    nc = _CACHE[key]

    in_maps = _prep_inputs(inputs, batch_lengths, W_in, W_state, b_state, W_proj)
    res = run_bass_kernel_spmd(nc, in_maps, core_ids=list(range(NCORES)))

    y_raw = res.results[0]["y"]          # [T, 128, 4, 64]
    y = y_raw.transpose(3, 0, 2, 1).reshape(64, T, 512)
    y = np.ascontiguousarray(y, dtype=np.float32)
    hf = res.results[0]["hf"]            # [128, 4, 64]
    h_fin = hf.transpose(2, 1, 0).reshape(64, 512)
    c_fin = np.empty((64, C), dtype=np.float32)
    for j in range(NCORES):
        cf = res.results[j]["cf"]
        c_fin[:, j * CLOC:(j + 1) * CLOC] = cf.transpose(2, 1, 0).reshape(64, 512)
    h_fin = np.ascontiguousarray(h_fin, dtype=np.float32)
    return y, (h_fin[None], c_fin[None])


# revision 4
# speedup vs baseline: 1.4813x; 1.4813x over previous
"""Trainium2 Bass kernel for LstmCellWithProjection (B=64,T=256,I=512,H=512,C=4096).

Strategy: 8-way tensor-parallel over the 4*C gate dimension (each core owns a
C/8=512 column slice of each of the i/f/g/o gates, so the cell state c stays
fully local). Everything on-chip lives in a [gate_rows_on_partitions, batch]
layout so no transposes are ever needed:

  - recurrent matmul: out[g,b] += W_stateT_chunk.T @ hT_chunk   (64 chunk MMs)
  - projection:       out[h,b] += W_projT_chunk.T @ s_chunk     (16 chunk MMs)
  - per-step bf16 AllReduce combines the 8 partial projections.

The input projection x @ W_in.T (+bias) is a separate big GEMM over (t,b)
blocks whose matmuls are interleaved 2 gate-tiles per step so the PE fills the
AllReduce latency window without ever delaying the serial recurrence chain;
results live in an SBUF ring (no DRAM staging).

Ragged lengths are specialized at trace time: every per-step op is sliced to
the active batch prefix A(t) (lengths are sorted descending), so finished
sequences freeze in place — their last h/c simply persist in the state tiles
and are DMA'd out at the end. y is only written for the active prefix; the
runtime zero-initializes outputs, so padding stays zero.
"""

import sys

sys.path.insert(0, "/opt/trn_rl_repo")

import numpy as np
import ml_dtypes

B, T, I, H, C = 64, 256, 512, 512, 4096
NCORES = 8
CLOC = C // NCORES            # 512 gate columns per gate per core
GT = 16                       # 16 gate tiles of 128 rows per core (4 gates x 4)
CLIP = 3.0

_CACHE = {}


def _build(Tmax, A):
    import concourse.bass as bass
    import concourse.tile as tile
    from concourse import bacc, mybir

    f32 = mybir.dt.float32
    bf16 = mybir.dt.bfloat16
    AF = mybir.ActivationFunctionType
    OP = mybir.AluOpType

    nc = bacc.Bacc("TRN2", target_bir_lowering=False, debug=False,
                   enable_asserts=False, num_devices=NCORES)

    wst_d = nc.dram_tensor("wst", [128, 64, 128], bf16, kind="ExternalInput")
    win_d = nc.dram_tensor("win", [128, 64, 128], bf16, kind="ExternalInput")
    wpt_d = nc.dram_tensor("wpt", [128, 16, 128], bf16, kind="ExternalInput")
    bias_d = nc.dram_tensor("bias", [128, 16], f32, kind="ExternalInput")
    xt_d = nc.dram_tensor("xt", [128, 4, T * 64], bf16, kind="ExternalInput")
    y_d = nc.dram_tensor("y", [T, 128, 4, 64], bf16, kind="ExternalOutput")
    hf_d = nc.dram_tensor("hf", [128, 4, 64], bf16, kind="ExternalOutput")
    cf_d = nc.dram_tensor("cf", [128, 4, 64], f32, kind="ExternalOutput")

    nblk = (Tmax + 7) // 8
    rings = {}
    xts = {}

    with tile.TileContext(nc) as tc:
        with (
            tc.tile_pool(name="wpool", bufs=1) as wpool,
            tc.tile_pool(name="state", bufs=1) as state,
            tc.tile_pool(name="ring", bufs=4) as ringp,
            tc.tile_pool(name="xtile", bufs=4) as xtp,
            tc.tile_pool(name="zb", bufs=2) as zb,
            tc.tile_pool(name="tmp", bufs=2) as tmp,
            tc.tile_pool(name="zps", bufs=1, space="PSUM") as zps,
            tc.tile_pool(name="pps", bufs=2, space="PSUM") as pps,
            tc.tile_pool(name="gps", bufs=2, space="PSUM") as gps,
            tc.tile_pool(name="drb", bufs=2, space="DRAM") as drb,
        ):
            wst = wpool.tile([128, 64, 128], bf16, tag="wst")
            win = wpool.tile([128, 64, 128], bf16, tag="win")
            wpt = wpool.tile([128, 16, 128], bf16, tag="wpt")
            bias = wpool.tile([128, 16], f32, tag="bias")
            nc.sync.dma_start(wst[:], wst_d[:])
            nc.sync.dma_start(win[:], win_d[:])
            nc.sync.dma_start(wpt[:], wpt_d[:])
            nc.sync.dma_start(bias[:], bias_d[:])

            c_sb = state.tile([128, 4, 64], f32, tag="c")
            hT = state.tile([128, 4, 64], bf16, tag="hT")
            hQ = state.tile([128, 4, 64], bf16, tag="hQ")
            nc.vector.memset(c_sb[:], 0.0)
            nc.vector.memset(hT[:], 0.0)
            nc.vector.memset(hQ[:], 0.0)

            def gemm_dma(blk):
                t0 = 8 * blk
                ncols = min(512, (Tmax - t0) * 64)
                rings[blk] = ringp.tile([128, 16, 512], bf16, tag="ring", name=f"ring{blk}")
                xt_t = xtp.tile([128, 4, 512], bf16, tag="xt", name=f"xt{blk}")
                xts[blk] = (xt_t, ncols)
                for k in range(4):
                    nc.sync.dma_start(xt_t[:, k, :ncols],
                                      xt_d[:, k, t0 * 64:t0 * 64 + ncols])

            def gemm_tiles(blk, gt_lo, gt_hi):
                xt_t, ncols = xts[blk]
                ring_t = rings[blk]
                for gt in range(gt_lo, gt_hi):
                    ps = gps.tile([128, 512], f32, tag="gps")
                    for k in range(4):
                        nc.tensor.matmul(ps[:, :ncols], win[:, k * 16 + gt, :],
                                         xt_t[:, k, :ncols],
                                         start=(k == 0), stop=(k == 3))
                    # epilogue on ScalarE: ring = psum + bias (Identity act)
                    nc.scalar.activation(ring_t[:, gt, :ncols], ps[:, :ncols],
                                         AF.Identity, bias=bias[:, gt:gt + 1])

            gemm_dma(0)
            gemm_tiles(0, 0, GT)
            if nblk > 1:
                gemm_dma(1)
                gemm_tiles(1, 0, GT)

            for t in range(Tmax):
                blk, dt = t // 8, t % 8
                fblk = blk + 2
                if fblk < nblk and dt == 0:
                    gemm_dma(fblk)

                At = A[t]
                ring_t = rings[blk]

                Z = zps.tile([128, 16, 64], f32, tag="Z")
                for gt in range(GT):
                    for k in range(4):
                        nc.tensor.matmul(Z[:, gt, :At], wst[:, k * 16 + gt, :],
                                         hT[:, k, :At],
                                         start=(k == 0), stop=(k == 3))

                # interleave 2 gate-tiles of the lookahead input GEMM; these
                # fill the PE while this step's AllReduce is in flight
                if fblk < nblk:
                    gemm_tiles(fblk, 2 * dt, 2 * dt + 2)

                z_sb = zb.tile([128, 16, 64], f32, tag="z")
                a_sb = zb.tile([128, 16, 64], f32, tag="a")
                rs = ring_t[:, :, dt * 64:dt * 64 + At]
                nc.vector.tensor_add(z_sb[:, 0:8, :At], Z[:, 0:8, :At],
                                     rs[:, 0:8, :])
                nc.vector.tensor_add(z_sb[:, 8:12, :At], Z[:, 8:12, :At],
                                     rs[:, 8:12, :])
                nc.vector.tensor_add(z_sb[:, 12:16, :At], Z[:, 12:16, :At],
                                     rs[:, 12:16, :])
                nc.scalar.activation(a_sb[:, 0:8, :At], z_sb[:, 0:8, :At],
                                     AF.Sigmoid)
                nc.scalar.activation(a_sb[:, 8:12, :At], z_sb[:, 8:12, :At],
                                     AF.Tanh)
                nc.scalar.activation(a_sb[:, 12:16, :At], z_sb[:, 12:16, :At],
                                     AF.Sigmoid)

                t1 = tmp.tile([128, 4, 64], f32, tag="t1")
                t2 = tmp.tile([128, 4, 64], f32, tag="t2")
                t3 = tmp.tile([128, 4, 64], f32, tag="t3")
                nc.vector.tensor_mul(t1[:, :, :At], a_sb[:, 0:4, :At],
                                     a_sb[:, 8:12, :At])
                nc.vector.tensor_mul(t2[:, :, :At], a_sb[:, 4:8, :At],
                                     c_sb[:, :, :At])
                nc.vector.tensor_add(t3[:, :, :At], t1[:, :, :At], t2[:, :, :At])
                nc.vector.tensor_scalar(c_sb[:, :, :At], t3[:, :, :At],
                                        CLIP, -CLIP, OP.min, OP.max)
                tc_t = tmp.tile([128, 4, 64], f32, tag="tc")
                nc.scalar.activation(tc_t[:, :, :At], c_sb[:, :, :At], AF.Tanh)
                s_bf = tmp.tile([128, 4, 64], bf16, tag="s")
                nc.vector.tensor_mul(s_bf[:, :, :At], a_sb[:, 12:16, :At],
                                     tc_t[:, :, :At])

                hP = pps.tile([128, 4, 64], f32, tag="hP")
                for m in range(4):
                    for k in range(4):
                        nc.tensor.matmul(hP[:, m, :At], wpt[:, k * 4 + m, :],
                                         s_bf[:, k, :At],
                                         start=(k == 0), stop=(k == 3))

                send = tmp.tile([128, 4, 64], bf16, tag="send")
                nc.scalar.copy(send[:, :, :At], hP[:, :, :At])
                bin_t = drb.tile([128, 4, At], bf16, tag="bin")
                bout_t = drb.tile([128, 4, At], bf16, tag="bout")
                nc.sync.dma_start(bin_t[:], send[:, :, :At])
                nc.gpsimd.collective_compute(
                    "AllReduce", mybir.AluOpType.add,
                    replica_groups=[list(range(NCORES))],
                    ins=[bin_t.opt()], outs=[bout_t.opt()],
                )
                recv = tmp.tile([128, 4, 64], bf16, tag="recv")
                nc.sync.dma_start(recv[:, :, :At], bout_t[:])

                # next-step matmul operand (critical path) on VectorE;
                # output/state copy on GpSimd
                nc.vector.tensor_scalar(hT[:, :, :At], recv[:, :, :At],
                                        CLIP, -CLIP, OP.min, OP.max)
                nc.gpsimd.tensor_scalar(hQ[:, :, :At], recv[:, :, :At],
                                        CLIP, -CLIP, OP.min, OP.max)
                nc.sync.dma_start(y_d[t, :, :, 0:At], hQ[:, :, 0:At])

            nc.sync.dma_start(hf_d[:], hQ[:])
            nc.sync.dma_start(cf_d[:], c_sb[:])

    nc.compile()
    return nc


def _prep_inputs(inputs, batch_lengths, W_in, W_state, b_state, W_proj):
    x = np.asarray(inputs, dtype=np.float32)
    W_in = np.asarray(W_in, dtype=np.float32)
    W_state = np.asarray(W_state, dtype=np.float32)
    b_state = np.asarray(b_state, dtype=np.float32)
    W_proj = np.asarray(W_proj, dtype=np.float32)

    # xt[p, k, t*64+b] = x[b, t, k*128+p]
    xt = x.transpose(2, 1, 0).reshape(4, 128, T * 64).transpose(1, 0, 2)
    xt = np.ascontiguousarray(xt).astype(ml_dtypes.bfloat16)

    in_maps = []
    for j in range(NCORES):
        rows = np.concatenate(
            [q * C + j * CLOC + np.arange(CLOC) for q in range(4)])
        wst_np = W_state[rows, :].T          # [512 h, 2048 g]
        wst = wst_np.reshape(4, 128, 16, 128).transpose(1, 0, 2, 3)
        win_np = W_in[rows, :].T
        win = win_np.reshape(4, 128, 16, 128).transpose(1, 0, 2, 3)
        bias = b_state[rows].reshape(16, 128).T   # [128, 16]
        wp_np = W_proj[:, j * CLOC:(j + 1) * CLOC].T   # [512 c, 512 h]
        wpt = wp_np.reshape(4, 128, 4, 128).transpose(1, 0, 2, 3)
        in_maps.append({
            "wst": np.ascontiguousarray(wst).astype(ml_dtypes.bfloat16),
            "win": np.ascontiguousarray(win).astype(ml_dtypes.bfloat16),
            "wpt": np.ascontiguousarray(wpt).astype(ml_dtypes.bfloat16),
            "bias": np.ascontiguousarray(bias, dtype=np.float32),
            "xt": xt,
        })
    return in_maps


def kernel(inputs, batch_lengths, W_in, W_state, b_state, W_proj):
    from concourse.bass_utils import run_bass_kernel_spmd

    lengths = np.asarray(batch_lengths).astype(np.int64)
    Tmax = int(lengths.max())
    A = [int((lengths > t).sum()) for t in range(Tmax + 1)]

    key = (Tmax, tuple(A))
    if key not in _CACHE:
        _CACHE[key] = _build(Tmax, A)
    nc = _CACHE[key]

    in_maps = _prep_inputs(inputs, batch_lengths, W_in, W_state, b_state, W_proj)
    res = run_bass_kernel_spmd(nc, in_maps, core_ids=list(range(NCORES)))

    y_raw = np.asarray(res.results[0]["y"]).astype(np.float32)  # [T,128,4,64]
    y = y_raw.transpose(3, 0, 2, 1).reshape(64, T, 512)
    y = np.ascontiguousarray(y, dtype=np.float32)
    hf = np.asarray(res.results[0]["hf"]).astype(np.float32)    # [128, 4, 64]
    h_fin = np.ascontiguousarray(hf.transpose(2, 1, 0).reshape(64, 512),
                                 dtype=np.float32)
    c_fin = np.empty((64, C), dtype=np.float32)
    for j in range(NCORES):
        cf = np.asarray(res.results[j]["cf"])
        c_fin[:, j * CLOC:(j + 1) * CLOC] = cf.transpose(2, 1, 0).reshape(64, 512)
    return y, (h_fin[None], c_fin[None])
